# revision 1
# baseline (speedup 1.0000x reference)
"""Bass/Tile MHA kernel for trn2 — builder + host shard/unshard helpers.

Per-core work (8 cores): core c handles batch b=c//2, head-group g=c%2
(8 of 16 heads). v2: head PAIRS share the qk_sb partition dim (head A at
partitions 0-47, head B at 64-111) so the QK^T matmuls are row-group
packed: two concurrent K=48 matmuls with M=128 (tile positions (0,0) and
(64,0)) per k-tile. Exact-K/exact-M matmuls mean no pad zeros anywhere
except the outT rows read by the final projection.

Dataflow (all matmuls bf16 in / fp32 PSUM accumulate):
  qkT[d_h, t]  = w_qk^T x           (pair-packed: rows 0-47 / 64-111)
  V[t, d_v]    = x w_v              (v_sb cols: 1 ones + 48 dims)
  S^T[k, q]    = (K^T)^T Q^T        (K=48, M=128, 2 heads row-packed)
  P^T          = exp(S^T)           (ScalarE true exp / VectorE fast-exp,
                                     split tunable per slot)
  outT'[d,q],l = (V|1)^T P^T        (M=49; ones col gives softmax denoms
                                     on PSUM rows 0 / 64, col packing)
  outT         = outT' * bcast(1/l)
  y[t, j]      = outT^T w_out       (+ b_out and cross-core sum on host)
"""

import math

import numpy as np
import ml_dtypes

import concourse.bass as bass
import concourse.mybir as mybir
import concourse.tile as tile
from concourse import bacc

F32 = mybir.dt.float32
BF16 = mybir.dt.bfloat16
I16 = mybir.dt.int16
AF = mybir.ActivationFunctionType
OP = mybir.AluOpType

DIM = 768
PH = 48
NP = 4          # head pairs per core
HC = 8          # heads per core
NDT = DIM // 128  # 6 contraction tiles for the projections

# Schraudolph fast-exp in bf16 bit space: bits = round(x*128/ln2 + (127*128 - C))
SCH_A = 128.0 / math.log(2.0)
SCH_C = 4.7
# +0.5: the fp32->int16 convert truncates, this re-centers it to round-nearest
SCH_B = 127.0 * 128.0 - SCH_C + 0.5

# exp engine per (kt*2 + hh) % 16 slot: 'V' = VectorE Schraudolph,
# 'S' = ScalarE true exp.  Roughly 50/50 keeps both engines level.
EXP_PATTERN = "SVVSVSVSVSVSVSVS"


def build_kernel(T=2048, exp_pattern=EXP_PATTERN, num_devices=8, qk32=False,
                 avg=8):
    KT = T // 128                 # k-tiles (token tiles)
    QCW = min(512, T)             # q chunk width (one PSUM bank)
    NQG = T // QCW                # q groups, one chunk each

    nc = bacc.Bacc("TRN2", target_bir_lowering=False, debug=False,
                   num_devices=num_devices)

    xt_d = nc.dram_tensor("xt", (DIM, T), BF16, kind="ExternalInput")
    wqk_d = nc.dram_tensor("wqk", (DIM, NP * 2 * 128), BF16, kind="ExternalInput")
    wv_d = nc.dram_tensor("wv", (DIM, HC * PH), BF16, kind="ExternalInput")
    wo_d = nc.dram_tensor("wo", (NP * 128, DIM), BF16, kind="ExternalInput")
    bqk_d = nc.dram_tensor("bqk", (128, NP * 2), F32, kind="ExternalInput")
    y_d = nc.dram_tensor("y", (T, DIM), F32, kind="ExternalOutput")

    with tile.TileContext(nc) as tc:
        with (
            tc.tile_pool(name="const", bufs=1) as cpool,
            tc.tile_pool(name="pt", bufs=34) as ptpool,
            tc.tile_pool(name="norm", bufs=2) as npool,
            tc.tile_pool(name="st", bufs=5, space="PSUM") as stpool,
            tc.tile_pool(name="av", bufs=1, space="PSUM") as avpool,
            tc.tile_pool(name="pj", bufs=2, space="PSUM") as pjpool,
        ):
            # ---- persistent SBUF tensors ----
            xt_sb = cpool.tile([128, NDT, T], BF16, tag="xt")
            wqk_sb = cpool.tile([128, NDT, NP * 2 * 128], BF16, tag="wqk")
            wv_sb = cpool.tile([128, NDT, HC * PH], BF16, tag="wv")
            wo_sb = cpool.tile([128, NP, DIM], BF16, tag="wo")
            bqk_sb = cpool.tile([128, NP * 2], F32, tag="bqk")
            # pair-packed Q^T/K^T: head 2p at partitions 0-47, head 2p+1 at
            # 64-111.  K=48 contraction reads only the live rows, so the
            # remaining partitions never need zeroing.
            qk_sb = cpool.tile([128, NP, 2, T], BF16, tag="qk")
            # V' columns per head: 0 = ones (softmax denominator lands on
            # PSUM row 0 / 64 of the shared bank), 1-48 = V
            v_sb = cpool.tile([128, KT, HC, PH + 1], BF16, tag="v")
            outT_sb = cpool.tile([128, NP, T], BF16, tag="outT")

            # ---- input DMAs (column-split so early chunks land fast) ----
            nc.sync.dma_start(bqk_sb[:], bqk_d[:])  # first: gates 1st proj copy
            # preload the exp activation table during the DMA wait
            warm = cpool.tile([1, 2], F32, tag="warm")
            nc.vector.memset(warm[0:1, 0:1], 0.0)
            nc.scalar.activation(warm[0:1, 1:2], warm[0:1, 0:1], AF.Exp)
            # first q-chunk of xt + pair-0 weights first: the earliest
            # qkT_proj matmuls only need these
            for dt_i in range(NDT):
                ts4 = np.s_[0:T // 4]
                nc.sync.dma_start(xt_sb[:, dt_i, ts4], xt_d[dt_i * 128:(dt_i + 1) * 128, ts4])
                nc.sync.dma_start(wqk_sb[:, dt_i, 0:256], wqk_d[dt_i * 128:(dt_i + 1) * 128, 0:256])
            for dt_i in range(NDT):
                for tc4 in range(1, 4):
                    ts4 = np.s_[tc4 * (T // 4):(tc4 + 1) * (T // 4)]
                    nc.sync.dma_start(xt_sb[:, dt_i, ts4], xt_d[dt_i * 128:(dt_i + 1) * 128, ts4])
                nc.sync.dma_start(wqk_sb[:, dt_i, 256:NP * 2 * 128], wqk_d[dt_i * 128:(dt_i + 1) * 128, 256:NP * 2 * 128])
                nc.sync.dma_start(wv_sb[:, dt_i, :], wv_d[dt_i * 128:(dt_i + 1) * 128, :])
            for p in range(NP):
                nc.sync.dma_start(wo_sb[:, p, :], wo_d[p * 128:(p + 1) * 128, :])

            # ones column for the softmax-denominator trick
            nc.vector.memset(v_sb[:, :, :, 0:1], 1.0)
            # outT pad rows (49-63, 113-127) are contracted against zero
            # rows of wo in final_proj; they only need to be finite.  32-
            # aligned bases required; rows 32-48 / 96-112 are overwritten
            # by every normalize.
            nc.vector.memset(outT_sb[32:64, :, :], 0.0)
            nc.gpsimd.memset(outT_sb[96:128, :, :], 0.0)

            def qkT_chunk(p, ci):
                # qkT[d_h, t]: rows 0-47 head A dims, 64-111 head B
                qk, tcI = divmod(ci, T // QCW)
                col0 = (p * 2 + qk) * 128
                ps = pjpool.tile([128, QCW], F32, tag="pj")
                for dt_i in range(NDT):
                    for mh in range(2):
                        nc.tensor.matmul(
                            ps[mh * 64:(mh + 1) * 64, :],
                            wqk_sb[:, dt_i, col0 + mh * 64:col0 + (mh + 1) * 64],
                            xt_sb[:, dt_i, tcI * QCW:(tcI + 1) * QCW],
                            start=(dt_i == 0), stop=(dt_i == NDT - 1),
                            skip_group_check=True)
                tsl = np.s_[tcI * QCW:(tcI + 1) * QCW]
                # one copy for both heads: rows 48-63 carry garbage
                # into qk_sb but K=48 matmuls never read them
                nc.scalar.activation(
                    qk_sb[0:64 + PH, p, qk, tsl], ps[0:64 + PH, :],
                    AF.Identity,
                    bias=bqk_sb[0:64 + PH, p * 2 + qk:p * 2 + qk + 1])

            def qkT_proj(p):
                for ci in range(2 * (T // QCW)):
                    qkT_chunk(p, ci)

            def v_proj_chunk(tt):
                psb = pjpool.tile([128, QCW], F32, tag="pj")
                ps = psb[:, 0:HC * PH]
                for dt_i in range(NDT):
                    for mh in range(2):
                        nc.tensor.matmul(
                            ps[mh * 64:(mh + 1) * 64, :],
                            xt_sb[:, dt_i, tt * 128 + mh * 64:tt * 128 + (mh + 1) * 64],
                            wv_sb[:, dt_i, :],
                            start=(dt_i == 0), stop=(dt_i == NDT - 1),
                            skip_group_check=True)
                nc.scalar.activation(
                    v_sb[:, tt, :, 1:PH + 1],
                    ps[:].rearrange("p (h d) -> p h d", h=HC),
                    AF.Copy)

            AVG = avg  # k-tiles per matmul-mode phase (QK vs AV tile configs)

            def attention(p, qg, av_extra=None):
                cs = np.s_[qg * QCW:(qg + 1) * QCW]
                av = avpool.tile([128, QCW], F32, tag="av", name="av")
                pts = {}

                def qk_emit(kt):
                    for hh in range(2):
                        b0 = hh * 64
                        st = stpool.tile([128, QCW], F32, tag="st")
                        if qk32:
                            # 32-granularity row tiles: split the K=48
                            # contraction into K=32 + K=16 accumulating
                            # matmuls in single-strip row groups
                            for c0, c1 in ((0, 32), (32, PH)):
                                nc.tensor.matmul(
                                    st[:],
                                    qk_sb[b0 + c0:b0 + c1, p, 1, kt * 128:(kt + 1) * 128],
                                    qk_sb[b0 + c0:b0 + c1, p, 0, cs],
                                    start=(c0 == 0), stop=(c0 != 0),
                                    tile_position=(b0 + c0, 0),
                                    skip_group_check=True)
                        else:
                            # row-packed: K=48 contraction in row-group hh,
                            # M=128 (full k-token tile)
                            nc.tensor.matmul(
                                st[:],
                                qk_sb[b0:b0 + PH, p, 1, kt * 128:(kt + 1) * 128],
                                qk_sb[b0:b0 + PH, p, 0, cs],
                                start=True, stop=True,
                                skip_group_check=True)
                        pt = ptpool.tile([128, QCW], BF16, tag="pt")
                        if exp_pattern[(kt * 2 + hh) % len(exp_pattern)] == "V":
                            nc.vector.tensor_scalar(
                                pt[:].bitcast(I16), st[:], SCH_A, SCH_B,
                                OP.mult, OP.add)
                        else:
                            nc.scalar.activation(pt[:], st[:], AF.Exp)
                        pts[(kt, hh)] = pt

                def av_emit(kt):
                    for hh in range(2):
                        nc.tensor.matmul(
                            av[hh * 64:hh * 64 + PH + 1, :],
                            v_sb[:, kt, p * 2 + hh, :],
                            pts.pop((kt, hh))[:],
                            start=(kt == 0), stop=(kt == KT - 1),
                            skip_group_check=True)

                # mode-batched phases: a group of AVG QK matmuls (64x128
                # tiles), then the previous group's AV matmuls (128x64
                # tiles).  PE array reconfig costs ~330ns, so amortize it.
                # av_extra(i) rides along after each AV phase (same tile
                # config) to interleave projection work for later calls.
                for g in range(KT // AVG):
                    for kt in range(g * AVG, (g + 1) * AVG):
                        qk_emit(kt)
                    if g > 0:
                        for kt in range((g - 1) * AVG, g * AVG):
                            av_emit(kt)
                        if av_extra is not None:
                            av_extra(g - 1)
                for kt in range(KT - AVG, KT):
                    av_emit(kt)
                if av_extra is not None:
                    av_extra(KT // AVG - 1)
                # normalize into outT (denominators live in rows 0 / 64).
                # partition_broadcast is only reliable with base-0 in/out APs,
                # so each head gets its own base-0 recip + broadcast tiles.
                r2a = npool.tile([128, QCW], F32, tag="r2", name="r2a")
                r2b = npool.tile([128, QCW], F32, tag="r2", name="r2b")
                rbca = npool.tile([128, QCW], F32, tag="rbc", name="rbca")
                rbcb = npool.tile([128, QCW], F32, tag="rbc", name="rbcb")
                lra = npool.tile([128, QCW], F32, tag="lr", name="lra")
                lrb = npool.tile([128, QCW], F32, tag="lr", name="lrb")
                nc.scalar.copy(lra[0:1, :], av[0:1, :])
                nc.scalar.copy(lrb[0:1, :], av[64:65, :])
                nc.vector.reciprocal_approx_fast(r2a[0:1, :], lra[0:1, :])
                nc.vector.reciprocal_approx_fast(r2b[0:1, :], lrb[0:1, :])
                nc.gpsimd.partition_broadcast(rbca[0:PH + 1, :], r2a[0:1, :])
                nc.gpsimd.partition_broadcast(rbcb[0:PH + 1, :], r2b[0:1, :])
                nc.vector.tensor_mul(outT_sb[0:PH + 1, p, cs],
                                     av[0:PH + 1, :], rbca[0:PH + 1, :])
                nc.vector.tensor_mul(outT_sb[64:64 + PH + 1, p, cs],
                                     av[64:64 + PH + 1, :], rbcb[0:PH + 1, :])

            def final_proj(qg):
                for tt in range(QCW // 128):
                    t0 = qg * QCW + tt * 128
                    ysb = npool.tile([128, DIM], F32, tag="ysb")
                    for jc in range(2):
                        js = np.s_[jc * 384:(jc + 1) * 384]
                        psb = pjpool.tile([128, QCW], F32, tag="pj", name=f"yp{jc}")
                        ps = psb[:, 0:384]
                        for p in range(NP):
                            for mh in range(2):
                                nc.tensor.matmul(
                                    ps[mh * 64:(mh + 1) * 64, :],
                                    outT_sb[:, p, t0 + mh * 64:t0 + (mh + 1) * 64],
                                    wo_sb[:, p, js],
                                    start=(p == 0), stop=(p == NP - 1),
                                    skip_group_check=True)
                        if (tt + jc) % 2 == 0:
                            nc.scalar.copy(ysb[:, js], ps[:])
                        else:
                            nc.vector.tensor_copy(ysb[:, js], ps[:])
                    nc.sync.dma_start(y_d[t0:t0 + 128, :], ysb[:])

            # ---- emission order (scheduling priority) ----
            qkT_proj(0)
            # v_proj for the k-tiles the first attention call consumes
            # immediately; the rest (and the next pair's qkT projection)
            # ride along inside the qg==0 attention calls' AV phases —
            # same PE tile config, so no extra mode switches, and the exp
            # engines stay fed while the PE does projection work.
            for tt in range(AVG):
                v_proj_chunk(tt)

            def hook_qg0(p):
                # i runs 0..KT//AVG-1 (4 calls per attention)
                def hook(i):
                    if p == 0:
                        for tt in range((i + 1) * AVG, (i + 2) * AVG):
                            if tt < KT:
                                v_proj_chunk(tt)
                    if p + 1 < NP:
                        n = 2 * (T // QCW)  # 8 chunks over 4 calls
                        per = (n + KT // AVG - 1) // (KT // AVG)
                        for ci in range(i * per, min((i + 1) * per, n)):
                            qkT_chunk(p + 1, ci)
                return hook

            for qg in range(NQG):
                for p in range(NP):
                    attention(p, qg,
                              av_extra=hook_qg0(p) if qg == 0 else None)
                    # emit the previous q-group's output projection here so
                    # its PE work fills the last pair's normalize latency
                    if p == 0 and qg > 0:
                        final_proj(qg - 1)
            final_proj(NQG - 1)

    nc.compile()
    return nc


# ---------------- host-side sharding ----------------

def host_prep(x, w_in, b_in, w_out, T=2048):
    """Full inputs -> list of 8 per-core input dicts."""
    scale = 1.0 / math.sqrt(PH)
    wr = np.asarray(w_in).reshape(DIM, 16, 3, PH)
    br = np.asarray(b_in).reshape(16, 3, PH)
    wog = np.asarray(w_out)  # (768, 768), row dv = h*48+d
    in_maps = []
    for c in range(8):
        b, g = divmod(c, 2)
        wqk = np.zeros((DIM, NP * 2 * 128), np.float32)
        bqk = np.zeros((128, NP * 2), np.float32)
        wv = np.zeros((DIM, HC * PH), np.float32)
        wo = np.zeros((NP * 128, DIM), np.float32)
        for p in range(NP):
            for hh, base in ((0, 0), (1, 64)):
                gh = g * 8 + p * 2 + hh
                wqk[:, (p * 2) * 128 + base:(p * 2) * 128 + base + PH] = wr[:, gh, 0] * scale
                wqk[:, (p * 2 + 1) * 128 + base:(p * 2 + 1) * 128 + base + PH] = wr[:, gh, 1]
                bqk[base:base + PH, p * 2] = br[gh, 0] * scale
                bqk[base:base + PH, p * 2 + 1] = br[gh, 1]
                wv[:, (p * 2 + hh) * PH:(p * 2 + hh + 1) * PH] = wr[:, gh, 2]
                wo[p * 128 + base + 1:p * 128 + base + 1 + PH, :] = wog[gh * PH:(gh + 1) * PH, :]
        in_maps.append({
            "xt": np.ascontiguousarray(np.asarray(x)[b].T).astype(ml_dtypes.bfloat16),
            "wqk": wqk.astype(ml_dtypes.bfloat16),
            "wv": wv.astype(ml_dtypes.bfloat16),
            "wo": wo.astype(ml_dtypes.bfloat16),
            "bqk": bqk,
        })
    return in_maps


def host_post(results, b_out, b_in, w_out, B=4, T=2048):
    # the V bias contributes bv @ w_out, a per-column constant: add on host
    bv_all = np.asarray(b_in).reshape(16, 3, PH)[:, 2, :].reshape(DIM)
    const = np.asarray(b_out) + bv_all @ np.asarray(w_out)
    out = np.empty((B, T, DIM), np.float32)
    for b in range(B):
        out[b] = results[2 * b]["y"] + results[2 * b + 1]["y"] + const[None, :]
    return out


# ---------------- self-contained kernel() entry point ----------------

_CACHED = {}


def _get_nc():
    if "nc" not in _CACHED:
        _CACHED["nc"] = build_kernel(T=2048, num_devices=8)
    return _CACHED["nc"]


def kernel(x, w_in, b_in, w_out, b_out):
    """Full-input MHA forward on 8 NeuronCores.

    x: (4, 2048, 768) f32; w_in: (768, 2304); b_in: (2304,);
    w_out: (768, 768); b_out: (768,). Returns (4, 2048, 768) f32.
    """
    from concourse.bass_utils import run_bass_kernel_spmd

    x = np.asarray(x, np.float32)
    w_in = np.asarray(w_in, np.float32)
    b_in = np.asarray(b_in, np.float32)
    w_out = np.asarray(w_out, np.float32)
    b_out = np.asarray(b_out, np.float32)

    nc = _get_nc()
    in_maps = host_prep(x, w_in, b_in, w_out, T=2048)
    res = run_bass_kernel_spmd(nc, in_maps, core_ids=list(range(8)))
    return host_post(res.results, b_out, b_in, w_out, B=4, T=2048)



# revision 8
# speedup vs baseline: 1.1385x; 1.1385x over previous
"""Bass/Tile MHA kernel for trn2 — builder + host shard/unshard helpers.

Per-core work (8 cores): core c handles batch b=c//2, head-group g=c%2
(8 of 16 heads). v2: head PAIRS share the qk_sb partition dim (head A at
partitions 0-47, head B at 64-111) so the QK^T matmuls are row-group
packed: two concurrent K=48 matmuls with M=128 (tile positions (0,0) and
(64,0)) per k-tile. Exact-K/exact-M matmuls mean no pad zeros anywhere
except the outT rows read by the final projection.

Dataflow (all matmuls bf16 in / fp32 PSUM accumulate):
  qkT[d_h, t]  = w_qk^T x           (pair-packed: rows 0-47 / 64-111)
  V[t, d_v]    = x w_v              (v_sb cols: 1 ones + 48 dims)
  S^T[k, q]    = (K^T)^T Q^T        (K=48, M=128, 2 heads row-packed)
  P^T          = exp(S^T)           (ScalarE true exp / VectorE fast-exp,
                                     split tunable per slot)
  outT'[d,q],l = (V|1)^T P^T        (M=49; ones col gives softmax denoms
                                     on PSUM rows 0 / 64, col packing)
  outT         = outT' * bcast(1/l)
  y[t, j]      = outT^T w_out       (+ b_out and cross-core sum on host)
"""

import math

import numpy as np
import ml_dtypes

import concourse.bass as bass
import concourse.mybir as mybir
import concourse.tile as tile
from concourse import bacc

F32 = mybir.dt.float32
BF16 = mybir.dt.bfloat16
I16 = mybir.dt.int16
AF = mybir.ActivationFunctionType
OP = mybir.AluOpType

DIM = 768
PH = 48
NP = 4          # head pairs per core
HC = 8          # heads per core
NDT = DIM // 128  # 6 contraction tiles for the projections

# Schraudolph fast-exp in bf16 bit space: bits = round(x*128/ln2 + (127*128 - C))
SCH_A = 128.0 / math.log(2.0)
SCH_C = 4.7
# +0.5: the fp32->int16 convert truncates, this re-centers it to round-nearest
SCH_B = 127.0 * 128.0 - SCH_C + 0.5

# exp engine per kt slot: 'V' = VectorE Schraudolph, 'S' = ScalarE true
# exp.  One [128, 2*QCW] exp op per kt (both heads batched, 2 PSUM
# banks).  S slightly oversubscribed: V also carries recips + muls.
EXP_PATTERN = "SVSVSVSSVSVSVSVS"


def build_kernel(T=2048, exp_pattern=EXP_PATTERN, num_devices=8, qk32=False,
                 avg=8):
    KT = T // 128                 # k-tiles (token tiles)
    QCW = min(512, T)             # q chunk width (one PSUM bank)
    NQG = T // QCW                # q groups, one chunk each

    nc = bacc.Bacc("TRN2", target_bir_lowering=False, debug=False,
                   num_devices=num_devices)

    xt_d = nc.dram_tensor("xt", (DIM, T), BF16, kind="ExternalInput")
    wqk_d = nc.dram_tensor("wqk", (DIM, NP * 2 * 128), BF16, kind="ExternalInput")
    wv_d = nc.dram_tensor("wv", (DIM, HC * PH), BF16, kind="ExternalInput")
    wo_d = nc.dram_tensor("wo", (NP * 128, DIM), BF16, kind="ExternalInput")
    bqk_d = nc.dram_tensor("bqk", (128, NP * 2), F32, kind="ExternalInput")
    y_d = nc.dram_tensor("y", (T, DIM), F32, kind="ExternalOutput")

    with tile.TileContext(nc) as tc:
        with (
            tc.tile_pool(name="const", bufs=1) as cpool,
            tc.tile_pool(name="pt", bufs=17) as ptpool,
            tc.tile_pool(name="norm", bufs=2) as npool,
            tc.tile_pool(name="st", bufs=2, space="PSUM") as stpool,
            tc.tile_pool(name="av", bufs=2, space="PSUM") as avpool,
            tc.tile_pool(name="pj", bufs=2, space="PSUM") as pjpool,
        ):
            # ---- persistent SBUF tensors ----
            xt_sb = cpool.tile([128, NDT, T], BF16, tag="xt")
            wqk_sb = cpool.tile([128, NDT, NP * 2 * 128], BF16, tag="wqk")
            wv_sb = cpool.tile([128, NDT, HC * PH], BF16, tag="wv")
            wo_sb = cpool.tile([128, NP, DIM], BF16, tag="wo")
            bqk_sb = cpool.tile([128, NP * 2], F32, tag="bqk")
            # pair-packed Q^T/K^T: head 2p at partitions 0-47, head 2p+1 at
            # 64-111.  K=48 contraction reads only the live rows, so the
            # remaining partitions never need zeroing.
            qk_sb = cpool.tile([128, NP, 2, T], BF16, tag="qk")
            # V' columns per head: 0 = ones (softmax denominator lands on
            # PSUM row 0 / 64 of the shared bank), 1-48 = V
            v_sb = cpool.tile([128, KT, HC, PH + 1], BF16, tag="v")
            outT_sb = cpool.tile([128, NP, T], BF16, tag="outT")

            # ---- input DMAs (column-split so early chunks land fast) ----
            nc.sync.dma_start(bqk_sb[:], bqk_d[:])  # first: gates 1st proj copy
            # preload the exp activation table during the DMA wait
            warm = cpool.tile([1, 2], F32, tag="warm")
            nc.vector.memset(warm[0:1, 0:1], 0.0)
            nc.scalar.activation(warm[0:1, 1:2], warm[0:1, 0:1], AF.Exp)
            # first q-chunk of xt + pair-0 weights first: the earliest
            # qkT_proj matmuls only need these
            for dt_i in range(NDT):
                ts4 = np.s_[0:T // 4]
                nc.sync.dma_start(xt_sb[:, dt_i, ts4], xt_d[dt_i * 128:(dt_i + 1) * 128, ts4])
                nc.sync.dma_start(wqk_sb[:, dt_i, 0:256], wqk_d[dt_i * 128:(dt_i + 1) * 128, 0:256])
            for dt_i in range(NDT):
                for tc4 in range(1, 4):
                    ts4 = np.s_[tc4 * (T // 4):(tc4 + 1) * (T // 4)]
                    nc.sync.dma_start(xt_sb[:, dt_i, ts4], xt_d[dt_i * 128:(dt_i + 1) * 128, ts4])
                nc.sync.dma_start(wqk_sb[:, dt_i, 256:NP * 2 * 128], wqk_d[dt_i * 128:(dt_i + 1) * 128, 256:NP * 2 * 128])
                nc.sync.dma_start(wv_sb[:, dt_i, :], wv_d[dt_i * 128:(dt_i + 1) * 128, :])
            for p in range(NP):
                nc.sync.dma_start(wo_sb[:, p, :], wo_d[p * 128:(p + 1) * 128, :])

            # ones column for the softmax-denominator trick
            nc.vector.memset(v_sb[:, :, :, 0:1], 1.0)
            # outT pad rows (49-63, 113-127) are contracted against zero
            # rows of wo in final_proj; they only need to be finite.  32-
            # aligned bases required; rows 32-48 / 96-112 are overwritten
            # by every normalize.
            nc.vector.memset(outT_sb[32:64, :, :], 0.0)
            nc.gpsimd.memset(outT_sb[96:128, :, :], 0.0)

            def qkT_chunk(p, ci):
                # qkT[d_h, t]: rows 0-47 head A dims, 64-111 head B
                qk, tcI = divmod(ci, T // QCW)
                col0 = (p * 2 + qk) * 128
                ps = pjpool.tile([128, QCW], F32, tag="pj")
                for dt_i in range(NDT):
                    for mh in range(2):
                        nc.tensor.matmul(
                            ps[mh * 64:(mh + 1) * 64, :],
                            wqk_sb[:, dt_i, col0 + mh * 64:col0 + (mh + 1) * 64],
                            xt_sb[:, dt_i, tcI * QCW:(tcI + 1) * QCW],
                            start=(dt_i == 0), stop=(dt_i == NDT - 1),
                            skip_group_check=True)
                tsl = np.s_[tcI * QCW:(tcI + 1) * QCW]
                # one copy for both heads: rows 48-63 carry garbage
                # into qk_sb but K=48 matmuls never read them
                nc.scalar.activation(
                    qk_sb[0:64 + PH, p, qk, tsl], ps[0:64 + PH, :],
                    AF.Identity,
                    bias=bqk_sb[0:64 + PH, p * 2 + qk:p * 2 + qk + 1])

            def qkT_proj(p):
                for ci in range(2 * (T // QCW)):
                    qkT_chunk(p, ci)

            def v_proj_chunk(tt):
                psb = pjpool.tile([128, QCW], F32, tag="pj")
                ps = psb[:, 0:HC * PH]
                for dt_i in range(NDT):
                    for mh in range(2):
                        nc.tensor.matmul(
                            ps[mh * 64:(mh + 1) * 64, :],
                            xt_sb[:, dt_i, tt * 128 + mh * 64:tt * 128 + (mh + 1) * 64],
                            wv_sb[:, dt_i, :],
                            start=(dt_i == 0), stop=(dt_i == NDT - 1),
                            skip_group_check=True)
                nc.scalar.activation(
                    v_sb[:, tt, :, 1:PH + 1],
                    ps[:].rearrange("p (h d) -> p h d", h=HC),
                    AF.Copy)

            AVG = avg  # k-tiles per matmul-mode phase (QK vs AV tile configs)

            def attention(p, qg, av_extra=None):
                cs = np.s_[qg * QCW:(qg + 1) * QCW]
                av = avpool.tile([128, QCW], F32, tag="av", name="av")
                pts = {}

                def qk_emit(kt):
                    # both heads of this k-tile into one 2-bank PSUM tile;
                    # a single [128, 2*QCW] exp op drains both (halves the
                    # S/V instruction count vs per-head exp ops)
                    st2 = stpool.tile([128, 2, QCW], F32, tag="st")
                    for hh in range(2):
                        b0 = hh * 64
                        # row-packed: K=48 contraction in row-group hh,
                        # M=128 (full k-token tile)
                        nc.tensor.matmul(
                            st2[:, hh, :],
                            qk_sb[b0:b0 + PH, p, 1, kt * 128:(kt + 1) * 128],
                            qk_sb[b0:b0 + PH, p, 0, cs],
                            start=True, stop=True,
                            skip_group_check=True)
                    pt2 = ptpool.tile([128, 2, QCW], BF16, tag="pt")
                    if exp_pattern[kt % len(exp_pattern)] == "V":
                        nc.vector.tensor_scalar(
                            pt2[:].bitcast(I16), st2[:], SCH_A, SCH_B,
                            OP.mult, OP.add)
                    else:
                        nc.scalar.activation(pt2[:], st2[:], AF.Exp)
                    pts[kt] = pt2

                def av_emit(kt):
                    pt2 = pts.pop(kt)
                    for hh in range(2):
                        nc.tensor.matmul(
                            av[hh * 64:hh * 64 + PH + 1, :],
                            v_sb[:, kt, p * 2 + hh, :],
                            pt2[:, hh, :],
                            start=(kt == 0), stop=(kt == KT - 1),
                            skip_group_check=True)

                # mode-batched phases: a group of AVG QK matmuls (64x128
                # tiles), then the previous group's AV matmuls (128x64
                # tiles).  PE array reconfig costs ~330ns, so amortize it.
                # av_extra(i) rides along after each AV phase (same tile
                # config) to interleave projection work for later calls.
                for g in range(KT // AVG):
                    for kt in range(g * AVG, (g + 1) * AVG):
                        qk_emit(kt)
                    if g > 0:
                        for kt in range((g - 1) * AVG, g * AVG):
                            av_emit(kt)
                        if av_extra is not None:
                            av_extra(g - 1)
                for kt in range(KT - AVG, KT):
                    av_emit(kt)
                if av_extra is not None:
                    av_extra(KT // AVG - 1)
                # normalize into outT (denominators live in rows 0 / 64).
                # partition_broadcast is only reliable with base-0 in/out APs,
                # so each head gets its own base-0 recip + broadcast tiles.
                r2a = npool.tile([128, QCW], F32, tag="r2", name="r2a")
                r2b = npool.tile([128, QCW], F32, tag="r2", name="r2b")
                rbca = npool.tile([128, QCW], F32, tag="rbc", name="rbca")
                rbcb = npool.tile([128, QCW], F32, tag="rbc", name="rbcb")
                lra = npool.tile([128, QCW], F32, tag="lr", name="lra")
                lrb = npool.tile([128, QCW], F32, tag="lr", name="lrb")
                nc.scalar.copy(lra[0:1, :], av[0:1, :])
                nc.scalar.copy(lrb[0:1, :], av[64:65, :])
                nc.vector.reciprocal_approx_fast(r2a[0:1, :], lra[0:1, :])
                nc.vector.reciprocal_approx_fast(r2b[0:1, :], lrb[0:1, :])
                nc.gpsimd.partition_broadcast(rbca[0:PH + 1, :], r2a[0:1, :])
                nc.gpsimd.partition_broadcast(rbcb[0:PH + 1, :], r2b[0:1, :])
                nc.vector.tensor_mul(outT_sb[0:PH + 1, p, cs],
                                     av[0:PH + 1, :], rbca[0:PH + 1, :])
                nc.vector.tensor_mul(outT_sb[64:64 + PH + 1, p, cs],
                                     av[64:64 + PH + 1, :], rbcb[0:PH + 1, :])

            def final_proj(qg, tts=None):
                for tt in (range(QCW // 128) if tts is None else tts):
                    t0 = qg * QCW + tt * 128
                    ysb = npool.tile([128, DIM], F32, tag="ysb")
                    for jc in range(2):
                        js = np.s_[jc * 384:(jc + 1) * 384]
                        psb = pjpool.tile([128, QCW], F32, tag="pj", name=f"yp{jc}")
                        ps = psb[:, 0:384]
                        for p in range(NP):
                            for mh in range(2):
                                nc.tensor.matmul(
                                    ps[mh * 64:(mh + 1) * 64, :],
                                    outT_sb[:, p, t0 + mh * 64:t0 + (mh + 1) * 64],
                                    wo_sb[:, p, js],
                                    start=(p == 0), stop=(p == NP - 1),
                                    skip_group_check=True)
                        if (tt + jc) % 2 == 0:
                            nc.scalar.copy(ysb[:, js], ps[:])
                        else:
                            nc.vector.tensor_copy(ysb[:, js], ps[:])
                    nc.sync.dma_start(y_d[t0:t0 + 128, :], ysb[:])

            # ---- emission order (scheduling priority) ----
            qkT_proj(0)
            # v_proj for the k-tiles the first attention call consumes
            # immediately; the rest (and the next pair's qkT projection)
            # ride along inside the qg==0 attention calls' AV phases —
            # same PE tile config, so no extra mode switches, and the exp
            # engines stay fed while the PE does projection work.
            for tt in range(AVG):
                v_proj_chunk(tt)

            def hook_qg0(p):
                # i runs 0..KT//AVG-1 (4 calls per attention)
                def hook(i):
                    if p == 0:
                        for tt in range((i + 1) * AVG, (i + 2) * AVG):
                            if tt < KT:
                                v_proj_chunk(tt)
                    if p + 1 < NP:
                        n = 2 * (T // QCW)  # 8 chunks over 4 calls
                        per = (n + KT // AVG - 1) // (KT // AVG)
                        for ci in range(i * per, min((i + 1) * per, n)):
                            qkT_chunk(p + 1, ci)
                return hook

            for qg in range(NQG):
                for p in range(NP):
                    attention(p, qg,
                              av_extra=hook_qg0(p) if qg == 0 else None)
                    # spread the previous q-group's output projection one
                    # token-tile per attention call: its PE work fills each
                    # call's normalize latency and keeps the array dense
                    if qg > 0:
                        final_proj(qg - 1, [p])
            final_proj(NQG - 1)

    nc.compile()
    return nc


# ---------------- host-side sharding ----------------

def host_prep(x, w_in, b_in, w_out, T=2048):
    """Full inputs -> list of 8 per-core input dicts."""
    scale = 1.0 / math.sqrt(PH)
    wr = np.asarray(w_in).reshape(DIM, 16, 3, PH)
    br = np.asarray(b_in).reshape(16, 3, PH)
    wog = np.asarray(w_out)  # (768, 768), row dv = h*48+d
    in_maps = []
    for c in range(8):
        b, g = divmod(c, 2)
        wqk = np.zeros((DIM, NP * 2 * 128), np.float32)
        bqk = np.zeros((128, NP * 2), np.float32)
        wv = np.zeros((DIM, HC * PH), np.float32)
        wo = np.zeros((NP * 128, DIM), np.float32)
        for p in range(NP):
            for hh, base in ((0, 0), (1, 64)):
                gh = g * 8 + p * 2 + hh
                wqk[:, (p * 2) * 128 + base:(p * 2) * 128 + base + PH] = wr[:, gh, 0] * scale
                wqk[:, (p * 2 + 1) * 128 + base:(p * 2 + 1) * 128 + base + PH] = wr[:, gh, 1]
                bqk[base:base + PH, p * 2] = br[gh, 0] * scale
                bqk[base:base + PH, p * 2 + 1] = br[gh, 1]
                wv[:, (p * 2 + hh) * PH:(p * 2 + hh + 1) * PH] = wr[:, gh, 2]
                wo[p * 128 + base + 1:p * 128 + base + 1 + PH, :] = wog[gh * PH:(gh + 1) * PH, :]
        in_maps.append({
            "xt": np.ascontiguousarray(np.asarray(x)[b].T).astype(ml_dtypes.bfloat16),
            "wqk": wqk.astype(ml_dtypes.bfloat16),
            "wv": wv.astype(ml_dtypes.bfloat16),
            "wo": wo.astype(ml_dtypes.bfloat16),
            "bqk": bqk,
        })
    return in_maps


def host_post(results, b_out, b_in, w_out, B=4, T=2048):
    # the V bias contributes bv @ w_out, a per-column constant: add on host
    bv_all = np.asarray(b_in).reshape(16, 3, PH)[:, 2, :].reshape(DIM)
    const = np.asarray(b_out) + bv_all @ np.asarray(w_out)
    out = np.empty((B, T, DIM), np.float32)
    for b in range(B):
        out[b] = results[2 * b]["y"] + results[2 * b + 1]["y"] + const[None, :]
    return out


# ---------------- self-contained kernel() entry point ----------------

_CACHED = {}


def _get_nc():
    if "nc" not in _CACHED:
        _CACHED["nc"] = build_kernel(T=2048, num_devices=8)
    return _CACHED["nc"]


def kernel(x, w_in, b_in, w_out, b_out):
    """Full-input MHA forward on 8 NeuronCores.

    x: (4, 2048, 768) f32; w_in: (768, 2304); b_in: (2304,);
    w_out: (768, 768); b_out: (768,). Returns (4, 2048, 768) f32.
    """
    from concourse.bass_utils import run_bass_kernel_spmd

    x = np.asarray(x, np.float32)
    w_in = np.asarray(w_in, np.float32)
    b_in = np.asarray(b_in, np.float32)
    w_out = np.asarray(w_out, np.float32)
    b_out = np.asarray(b_out, np.float32)

    nc = _get_nc()
    in_maps = host_prep(x, w_in, b_in, w_out, T=2048)
    res = run_bass_kernel_spmd(nc, in_maps, core_ids=list(range(8)))
    return host_post(res.results, b_out, b_in, w_out, B=4, T=2048)

